# revision 21
# baseline (speedup 1.0000x reference)
"""Trainium2 Bass kernel for nn_Attention_13778255085616.

Head-parallel tensor sharding across 8 NeuronCores (2 heads per core):
  - per-core QKV projection restricted to its heads' columns (q|k|v, 128 cols each)
  - streaming attention over the 16896-token KV cache in S^T layout:
      S^T[l, q] = K @ Q^T  (contraction D=64), exp fused on ACT (no max
      subtraction -- scores are bounded; softmax is shift-free here),
      AV accumulated in O^T layout with a ones-column on V giving the
      softmax denominators for free.
  - K^T tiles are produced by a grid-swapped DMA (two 32-column halves)
    followed by a DVE StreamTranspose (32x32 blocks), yielding an exact
    fp32 transpose without touching the TensorEngine.
  - per-head normalize via DVE reciprocal + GPSIMD partition_broadcast.
  - on-device output projection with W_proj rows for the core's heads;
    the 8 partial [512,1024] outputs are summed on host (the all-reduce),
    b_proj added once on host.
  - probe outputs are rows of the main (normalized, pre-projection)
    attention output, so they are gathered on host from the returned
    per-head O^T -- no separate probe compute on device.
"""
import numpy as np

import concourse.bass as bass
import concourse.mybir as mybir
import concourse.tile as tile
from concourse import bacc
from concourse.bass_utils import run_bass_kernel_spmd
from concourse.masks import make_identity

FP = mybir.dt.float32
FPR = mybir.dt.float32r
I32 = mybir.dt.int32
ActF = mybir.ActivationFunctionType

H, NQ, C, D, LP = 16, 512, 1024, 64, 16384
HPC = 2                      # heads per core
NCORES = 8
L = LP + NQ                  # 16896 total KV length
SCALE = float(D) ** -0.5
CH = 512                     # KV rows per DMA chunk (4 subchunks)
SUB = 128                    # KV rows per matmul subchunk
GRP = 2                      # subchunks per exp (ACT) group (PSUM banks)
NSUB = L // SUB              # 132
NGRP = NSUB // GRP           # 44
NPCH = LP // CH              # 32 past chunks per head
PF = 3                       # chunk prefetch distance

_cache = {}


def _build_program():
    nc = bacc.Bacc("TRN2", debug=False)
    x_d = nc.dram_tensor("x", [C, NQ], FPR, kind="ExternalInput")  # x^T (host)
    wq_d = nc.dram_tensor("wq", [3, 128, 8 * 128], FPR, kind="ExternalInput")
    bq_d = nc.dram_tensor("bq", [3 * 128], FP, kind="ExternalInput")
    wp_d = nc.dram_tensor("wp", [128, C], FPR, kind="ExternalInput")
    pk_d = nc.dram_tensor("pk", [HPC, D, LP], FPR, kind="ExternalInput")
    pv_d = nc.dram_tensor("pv", [HPC, LP, D + 1], FPR, kind="ExternalInput")
    pout_d = nc.dram_tensor("pout", [NQ, C], FP, kind="ExternalOutput")
    obar_d = nc.dram_tensor("obar", [HPC * D, NQ], FPR, kind="ExternalOutput")

    with tile.TileContext(nc) as tc:
        with (
            tc.tile_pool(name="const", bufs=1) as cpool,
            tc.tile_pool(name="kv", bufs=PF + 1) as kv_pool,
            tc.tile_pool(name="at", bufs=3) as at_pool,
            tc.tile_pool(name="outp", bufs=2) as out_pool,
            tc.tile_pool(name="psum_s", bufs=3, space="PSUM") as psum_s,
            tc.tile_pool(name="psum_o", bufs=2, space="PSUM") as psum_o,
        ):
            # ---------------- setup: weights / x^T / QKV^T ----------------
            ident = cpool.tile([128, 128], FP, tag="ident")
            make_identity(nc, ident)

            wqt = []
            for ocb in range(3):  # q | k | v column blocks (128 each)
                w = cpool.tile([128, 8 * 128], FPR, tag=f"wqt{ocb}")
                nc.sync.dma_start(out=w[:, :], in_=wq_d[ocb])
                wqt.append(w)
            bq_t = cpool.tile([128, 3], FP, tag="bqt")
            nc.sync.dma_start(out=bq_t[:, :],
                              in_=bq_d.rearrange("(ocb p) -> p ocb", p=128))
            wp_h = []
            for h in range(HPC):
                w = cpool.tile([64, C], FPR, tag=f"wp{h}")
                nc.sync.dma_start(out=w[:, :], in_=wp_d[64 * h:64 * (h + 1), :])
                wp_h.append(w)

            # x^T loaded directly (host pre-transposed)
            xT = []
            for cc in range(8):
                xt = cpool.tile([128, NQ], FPR, tag=f"xT{cc}")
                nc.sync.dma_start(out=xt[:, :],
                                  in_=x_d[128 * cc:128 * (cc + 1), :])
                xT.append(xt)

            # explicit persistent rings (deterministic round-robin; pool
            # slot allocation is LIFO and would thrash one slot).
            # NRING must exceed prefetch distance + AV lag window: chunk c's
            # slot is re-written by chunk c+NRING (emitted at group 2(c+NRING-PF))
            # and the lag-2 AV reads of chunk c are emitted at group 2c+3.
            NRING = PF + 3
            u_ring = [kv_pool.tile([64, CH], FPR, name=f"ur{i}", bufs=1, tag=f"u{i}")
                      for i in range(NRING)]
            vp_ring = [kv_pool.tile([128, 4 * 65], FPR, name=f"vpr{i}", bufs=1, tag=f"vp{i}")
                       for i in range(NRING)]
            sg_ring = [psum_s.tile([128, GRP * NQ], FP, name=f"sgr{i}", bufs=1, tag=f"sgr{i}")
                       for i in range(3)]
            at_ring = [at_pool.tile([128, GRP * NQ], FPR, name=f"atr{i}", bufs=1, tag=f"atr{i}")
                       for i in range(3)]


            # QKV^T: per output-column block, accumulate 8 C-chunks, then
            # split PSUM rows into per-head [64, 512] SBUF tiles (+bias)
            qT, kTn, vTn = [], [], []
            for ocb in range(3):
                ps = sg_ring[ocb][:, 0:NQ]
                for cc in range(8):
                    nc.tensor.matmul(ps, (wqt[ocb][:, 128 * cc:128 * (cc + 1)]),
                                     (xT[cc][:, :]), start=(cc == 0), stop=(cc == 7))
                for h in range(HPC):
                    t = cpool.tile([64, NQ], FPR if ocb < 2 else FP,
                                   tag=f"qkvT{ocb}{h}")
                    nc.scalar.activation(t[:, :], ps[64 * h:64 * (h + 1), 0:NQ],
                                         ActF.Identity,
                                         bias=bq_t[64 * h:64 * (h + 1),
                                                   ocb:ocb + 1])
                    [qT, kTn, vTn][ocb].append(t)

            # v_new in natural [L, D] layout with ones column: PE transpose
            vnew = []
            for h in range(HPC):
                vn = cpool.tile([128, 4 * 65], FPR, tag=f"vnew{h}")
                nc.vector.memset(
                    vn[:, :].rearrange("p (n j) -> p n j", n=4)[:, :, 64]
                    .bitcast(FP), 1.0)
                for c4 in range(4):
                    pt = sg_ring[h][0:128, 64 * c4:64 * (c4 + 1)]
                    nc.tensor.transpose(pt,
                                        vTn[h][:, 128 * c4:128 * (c4 + 1)],
                                        ident[0:64, 0:64])
                    nc.vector.tensor_copy(vn[:, 65 * c4:65 * c4 + 64], pt)
                vnew.append(vn)

            # ---------------- main attention, heads sequential ----------------
            obars = []
            for h in range(HPC):
                U_tiles = {}
                vp_tiles = {}

                def emit_chunk(c, h=h, U_tiles=U_tiles, vp_tiles=vp_tiles):
                    # K^T chunk: plain contiguous load (host pre-transposed)
                    u = u_ring[c % NRING]
                    nc.sync.dma_start(out=u[:, :],
                                      in_=pk_d[h, :, CH * c:CH * (c + 1)])
                    U_tiles[c] = u
                    # V' chunk (host pre-padded with ones column): full-tile write
                    vp = vp_ring[c % NRING]
                    dst = vp[:, :].rearrange("p (n j) -> p n j", n=4)
                    src = pv_d[h, CH * c:CH * (c + 1), :].rearrange(
                        "(n i) j -> i n j", n=4)
                    nc.sync.dma_start(out=dst, in_=src)
                    vp_tiles[c] = vp

                def k_lhsT(s):
                    if s < NPCH * 4:
                        return U_tiles[s // 4][:, 128 * (s % 4):128 * (s % 4 + 1)]
                    sn = s - NPCH * 4
                    return kTn[h][:, 128 * sn:128 * (sn + 1)]

                def v_lhsT(s):
                    if s < NPCH * 4:
                        return vp_tiles[s // 4][:, 65 * (s % 4):65 * (s % 4 + 1)]
                    sn = s - NPCH * 4
                    return vnew[h][:, 65 * sn:65 * sn + 65]

                for c in range(PF):
                    emit_chunk(c)
                o_ps = psum_o.tile([65, NQ], FP, tag="ot")

                def emit_av(g, at):
                    for m in range(GRP):
                        s = GRP * g + m
                        nc.tensor.matmul(o_ps[:, :], (v_lhsT(s)),
                                         (at[:, NQ * m:NQ * (m + 1)]),
                                         start=(s == 0), stop=(s == NSUB - 1))
                    done = (GRP * (g + 1)) // 4
                    for c in list(U_tiles):
                        if c < done - 1:
                            U_tiles.pop(c, None)
                            vp_tiles.pop(c, None)

                pend = []  # (g, at-tile) pending AV groups, lag 2
                for g in range(NGRP):
                    sg = sg_ring[g % 3]
                    for m in range(GRP):
                        s = GRP * g + m
                        if s % 4 == 0 and s // 4 + PF < NPCH:
                            emit_chunk(s // 4 + PF)
                        nc.tensor.matmul(sg[:, NQ * m:NQ * (m + 1)], (k_lhsT(s)),
                                         (qT[h][:, :]), start=True, stop=True)
                    if len(pend) >= 2:
                        emit_av(*pend.pop(0))
                    at = at_ring[g % 3]
                    nc.scalar.activation(at[:, :], sg[:, :], ActF.Exp, scale=SCALE)
                    pend.append((g, at))
                for p in pend:
                    emit_av(*p)

                # normalize: obar = O / s (broadcast s first, then a
                # partition-parallel reciprocal -- a [1,512] DVE recip is ~3.4us)
                s_sb = out_pool.tile([1, NQ], FP, tag="r")
                nc.vector.tensor_copy(s_sb[:, :], o_ps[64:65, :])
                s_bc = out_pool.tile([64, NQ], FP, tag="sbc")
                nc.gpsimd.partition_broadcast(s_bc[:, :], s_sb[:, :])
                r_bc = out_pool.tile([64, NQ], FP, tag="rbc")
                nc.vector.reciprocal(r_bc[:, :], s_bc[:, :])
                ob = cpool.tile([64, NQ], FPR, tag=f"obar{h}")
                nc.vector.tensor_mul(ob[:, :], o_ps[0:64, :], r_bc[:, :])
                nc.sync.dma_start(out=obar_d[64 * h:64 * (h + 1), :], in_=ob[:, :])
                obars.append(ob)

            # ---------------- output projection (partial, no bias) ----------------
            for qb in range(4):
                for nh in range(2):
                    pp = psum_o.tile([128, 512], FP, tag="ot")
                    for h in range(HPC):
                        nc.tensor.matmul(pp[:, :],
                                         (obars[h][:, 128 * qb:128 * (qb + 1)]),
                                         (wp_h[h][:, 512 * nh:512 * (nh + 1)]),
                                         start=(h == 0), stop=(h == HPC - 1))
                    ot = out_pool.tile([128, 512], FP, tag="osb")
                    if nh == 0:
                        nc.vector.tensor_copy(ot[:, :], pp[:, :])
                    else:
                        nc.scalar.copy(ot[:, :], pp[:, :])
                    nc.sync.dma_start(
                        out=pout_d[128 * qb:128 * (qb + 1),
                                   512 * nh:512 * (nh + 1)],
                        in_=ot[:, :])

    nc.compile()
    return nc


def _get_nc():
    if "nc" not in _cache:
        _cache["nc"] = _build_program()
    return _cache["nc"]


def kernel(x, past_k, past_v, W_qkv, b_qkv, W_proj, b_proj, probe_idx, **kw):
    x = np.asarray(x)
    past_k = np.asarray(past_k)
    past_v = np.asarray(past_v)
    W_qkv = np.asarray(W_qkv)
    b_qkv = np.asarray(b_qkv)
    W_proj = np.asarray(W_proj)
    b_proj = np.asarray(b_proj)
    probe_idx = np.asarray(probe_idx)

    nc = _get_nc()
    x2 = np.ascontiguousarray(x.reshape(NQ, C).T)
    pkT_all = np.ascontiguousarray(past_k[0].transpose(0, 2, 1))
    pvp_all = np.ascontiguousarray(np.concatenate(
        [past_v[0], np.ones((H, LP, 1), np.float32)], axis=2))
    in_maps = []
    for g in range(NCORES):
        cols = slice(128 * g, 128 * (g + 1))
        # [3, 128(C within chunk), 8*128(cc, oc)]: wq[ocb, p, cc*128+oc]
        # = W_qkv[cc*128+p, ocb*C + g*128 + oc]
        wq_sl = np.stack([
            W_qkv[:, ocb * C + 128 * g: ocb * C + 128 * (g + 1)]
            .reshape(8, 128, 128).transpose(1, 0, 2).reshape(128, 1024)
            for ocb in range(3)])
        bq_sl = np.concatenate(
            [b_qkv[cols], b_qkv[C:][cols], b_qkv[2 * C:][cols]])
        in_maps.append({
            "x": x2,
            "wq": np.ascontiguousarray(wq_sl),
            "bq": np.ascontiguousarray(bq_sl),
            "wp": np.ascontiguousarray(W_proj[cols, :]),
            "pk": pkT_all[2 * g:2 * g + 2],
            "pv": pvp_all[2 * g:2 * g + 2],
        })

    res = run_bass_kernel_spmd(nc, in_maps, core_ids=list(range(NCORES)),
                               trace=bool(_cache.get("trace", False)))
    _cache["last_results"] = res

    out = np.zeros((NQ, C), dtype=np.float32)
    for g in range(NCORES):
        out += res.results[g]["pout"]
    out += b_proj[None, :]

    o_probe = np.empty((1, H, len(probe_idx), D), dtype=np.float32)
    for g in range(NCORES):
        ob = res.results[g]["obar"]          # [128, 512]: rows 64h+d
        for j in range(HPC):
            oT = ob[64 * j:64 * (j + 1), :]  # [D, NQ]
            o_probe[0, 2 * g + j] = oT[:, probe_idx].T
    return out.reshape(1, NQ, C), o_probe


# revision 22
# speedup vs baseline: 1.0244x; 1.0244x over previous
"""Trainium2 Bass kernel for nn_Attention_13778255085616.

Head-parallel tensor sharding across 8 NeuronCores (2 heads per core):
  - per-core QKV projection restricted to its heads' columns (q|k|v, 128 cols each)
  - streaming attention over the 16896-token KV cache in S^T layout:
      S^T[l, q] = K @ Q^T  (contraction D=64), exp fused on ACT (no max
      subtraction -- scores are bounded; softmax is shift-free here),
      AV accumulated in O^T layout with a ones-column on V giving the
      softmax denominators for free.
  - K^T tiles are produced by a grid-swapped DMA (two 32-column halves)
    followed by a DVE StreamTranspose (32x32 blocks), yielding an exact
    fp32 transpose without touching the TensorEngine.
  - per-head normalize via DVE reciprocal + GPSIMD partition_broadcast.
  - on-device output projection with W_proj rows for the core's heads;
    the 8 partial [512,1024] outputs are summed on host (the all-reduce),
    b_proj added once on host.
  - probe outputs are rows of the main (normalized, pre-projection)
    attention output, so they are gathered on host from the returned
    per-head O^T -- no separate probe compute on device.
"""
import numpy as np

import concourse.bass as bass
import concourse.mybir as mybir
import concourse.tile as tile
from concourse import bacc
from concourse.bass_utils import run_bass_kernel_spmd
from concourse.masks import make_identity

FP = mybir.dt.float32
FPR = mybir.dt.float32r
I32 = mybir.dt.int32
ActF = mybir.ActivationFunctionType

H, NQ, C, D, LP = 16, 512, 1024, 64, 16384
HPC = 2                      # heads per core
NCORES = 8
L = LP + NQ                  # 16896 total KV length
SCALE = float(D) ** -0.5
CH = 512                     # KV rows per DMA chunk (4 subchunks)
SUB = 128                    # KV rows per matmul subchunk
GRP = 2                      # subchunks per exp (ACT) group (PSUM banks)
NSUB = L // SUB              # 132
NGRP = NSUB // GRP           # 44
NPCH = LP // CH              # 32 past chunks per head
PF = 3                       # chunk prefetch distance

_cache = {}


def _build_program():
    nc = bacc.Bacc("TRN2", debug=False)
    x_d = nc.dram_tensor("x", [C, NQ], FPR, kind="ExternalInput")  # x^T (host)
    wq_d = nc.dram_tensor("wq", [3, 128, 8 * 128], FPR, kind="ExternalInput")
    bq_d = nc.dram_tensor("bq", [3 * 128], FP, kind="ExternalInput")
    wp_d = nc.dram_tensor("wp", [128, C], FPR, kind="ExternalInput")
    pk_d = nc.dram_tensor("pk", [HPC, D, LP], FPR, kind="ExternalInput")
    pv_d = nc.dram_tensor("pv", [HPC, LP, D + 1], FPR, kind="ExternalInput")
    pout_d = nc.dram_tensor("pout", [NQ, C], FP, kind="ExternalOutput")
    obar_d = nc.dram_tensor("obar", [HPC * D, NQ], FPR, kind="ExternalOutput")

    with tile.TileContext(nc) as tc:
        with (
            tc.tile_pool(name="const", bufs=1) as cpool,
            tc.tile_pool(name="kv", bufs=PF + 1) as kv_pool,
            tc.tile_pool(name="at", bufs=3) as at_pool,
            tc.tile_pool(name="outp", bufs=2) as out_pool,
            tc.tile_pool(name="psum_s", bufs=3, space="PSUM") as psum_s,
            tc.tile_pool(name="psum_o", bufs=2, space="PSUM") as psum_o,
        ):
            # ---------------- setup: weights / x^T / QKV^T ----------------
            ident = cpool.tile([128, 128], FP, tag="ident")
            make_identity(nc, ident)

            # critical-path DMAs first: wqt[0] + x^T feed the first QKV matmuls
            wqt = []
            w0 = cpool.tile([128, 8 * 128], FPR, tag="wqt0")
            nc.sync.dma_start(out=w0[:, :], in_=wq_d[0])
            wqt.append(w0)
            xT = []
            for cc in range(8):
                xt = cpool.tile([128, NQ], FPR, tag=f"xT{cc}")
                nc.sync.dma_start(out=xt[:, :],
                                  in_=x_d[128 * cc:128 * (cc + 1), :])
                xT.append(xt)
            for ocb in (1, 2):
                w = cpool.tile([128, 8 * 128], FPR, tag=f"wqt{ocb}")
                nc.sync.dma_start(out=w[:, :], in_=wq_d[ocb])
                wqt.append(w)
            bq_t = cpool.tile([128, 3], FP, tag="bqt")
            nc.sync.dma_start(out=bq_t[:, :],
                              in_=bq_d.rearrange("(ocb p) -> p ocb", p=128))
            wp_h = []
            for h in range(HPC):
                w = cpool.tile([64, C], FPR, tag=f"wp{h}")
                nc.sync.dma_start(out=w[:, :], in_=wp_d[64 * h:64 * (h + 1), :])
                wp_h.append(w)

            # explicit persistent rings (deterministic round-robin; pool
            # slot allocation is LIFO and would thrash one slot).
            # NRING must exceed prefetch distance + AV lag window: chunk c's
            # slot is re-written by chunk c+NRING (emitted at group 2(c+NRING-PF))
            # and the lag-2 AV reads of chunk c are emitted at group 2c+3.
            NRING = PF + 3
            u_ring = [kv_pool.tile([64, CH], FPR, name=f"ur{i}", bufs=1, tag=f"u{i}")
                      for i in range(NRING)]
            vp_ring = [kv_pool.tile([128, 4 * 65], FPR, name=f"vpr{i}", bufs=1, tag=f"vp{i}")
                       for i in range(NRING)]
            sg_ring = [psum_s.tile([128, GRP * NQ], FP, name=f"sgr{i}", bufs=1, tag=f"sgr{i}")
                       for i in range(3)]
            at_ring = [at_pool.tile([128, GRP * NQ], FPR, name=f"atr{i}", bufs=1, tag=f"atr{i}")
                       for i in range(3)]


            # QKV^T: per output-column block, accumulate 8 C-chunks, then
            # split PSUM rows into per-head [64, 512] SBUF tiles (+bias)
            qT, kTn, vTn = [], [], []
            for ocb in range(3):
                ps = sg_ring[ocb][:, 0:NQ]
                for cc in range(8):
                    nc.tensor.matmul(ps, (wqt[ocb][:, 128 * cc:128 * (cc + 1)]),
                                     (xT[cc][:, :]), start=(cc == 0), stop=(cc == 7))
                for h in range(HPC):
                    t = cpool.tile([64, NQ], FPR if ocb < 2 else FP,
                                   tag=f"qkvT{ocb}{h}")
                    nc.scalar.activation(t[:, :], ps[64 * h:64 * (h + 1), 0:NQ],
                                         ActF.Identity,
                                         bias=bq_t[64 * h:64 * (h + 1),
                                                   ocb:ocb + 1])
                    [qT, kTn, vTn][ocb].append(t)

            # warm head-0 K/V prefetch: overlap with the rest of setup
            for c in range(PF):
                nc.sync.dma_start(out=u_ring[c][:, :],
                                  in_=pk_d[0, :, CH * c:CH * (c + 1)])
                nc.sync.dma_start(
                    out=vp_ring[c][:, :].rearrange("p (n j) -> p n j", n=4),
                    in_=pv_d[0, CH * c:CH * (c + 1), :].rearrange(
                        "(n i) j -> i n j", n=4))

            # v_new in natural [L, D] layout with ones column: PE transpose
            vnew = []
            for h in range(HPC):
                vn = cpool.tile([128, 4 * 65], FPR, tag=f"vnew{h}")
                nc.vector.memset(
                    vn[:, :].rearrange("p (n j) -> p n j", n=4)[:, :, 64]
                    .bitcast(FP), 1.0)
                for c4 in range(4):
                    pt = sg_ring[h][0:128, 64 * c4:64 * (c4 + 1)]
                    nc.tensor.transpose(pt,
                                        vTn[h][:, 128 * c4:128 * (c4 + 1)],
                                        ident[0:64, 0:64])
                    nc.vector.tensor_copy(vn[:, 65 * c4:65 * c4 + 64], pt)
                vnew.append(vn)

            # ---------------- main attention, heads sequential ----------------
            obars = []
            for h in range(HPC):
                U_tiles = {}
                vp_tiles = {}

                def emit_chunk(c, h=h, U_tiles=U_tiles, vp_tiles=vp_tiles):
                    # K^T chunk: plain contiguous load (host pre-transposed)
                    u = u_ring[c % NRING]
                    nc.sync.dma_start(out=u[:, :],
                                      in_=pk_d[h, :, CH * c:CH * (c + 1)])
                    U_tiles[c] = u
                    # V' chunk (host pre-padded with ones column): full-tile write
                    vp = vp_ring[c % NRING]
                    dst = vp[:, :].rearrange("p (n j) -> p n j", n=4)
                    src = pv_d[h, CH * c:CH * (c + 1), :].rearrange(
                        "(n i) j -> i n j", n=4)
                    nc.sync.dma_start(out=dst, in_=src)
                    vp_tiles[c] = vp

                def k_lhsT(s):
                    if s < NPCH * 4:
                        return U_tiles[s // 4][:, 128 * (s % 4):128 * (s % 4 + 1)]
                    sn = s - NPCH * 4
                    return kTn[h][:, 128 * sn:128 * (sn + 1)]

                def v_lhsT(s):
                    if s < NPCH * 4:
                        return vp_tiles[s // 4][:, 65 * (s % 4):65 * (s % 4 + 1)]
                    sn = s - NPCH * 4
                    return vnew[h][:, 65 * sn:65 * sn + 65]

                if h == 0:
                    for c in range(PF):   # DMAs already issued during setup
                        U_tiles[c] = u_ring[c % NRING]
                        vp_tiles[c] = vp_ring[c % NRING]
                else:
                    for c in range(PF):
                        emit_chunk(c)
                o_ps = psum_o.tile([65, NQ], FP, tag="ot")

                def emit_av(g, at):
                    for m in range(GRP):
                        s = GRP * g + m
                        nc.tensor.matmul(o_ps[:, :], (v_lhsT(s)),
                                         (at[:, NQ * m:NQ * (m + 1)]),
                                         start=(s == 0), stop=(s == NSUB - 1))
                    done = (GRP * (g + 1)) // 4
                    for c in list(U_tiles):
                        if c < done - 1:
                            U_tiles.pop(c, None)
                            vp_tiles.pop(c, None)

                pend = []  # (g, at-tile) pending AV groups, lag 2
                for g in range(NGRP):
                    sg = sg_ring[g % 3]
                    for m in range(GRP):
                        s = GRP * g + m
                        if s % 4 == 0 and s // 4 + PF < NPCH:
                            emit_chunk(s // 4 + PF)
                        nc.tensor.matmul(sg[:, NQ * m:NQ * (m + 1)], (k_lhsT(s)),
                                         (qT[h][:, :]), start=True, stop=True)
                    if len(pend) >= 2:
                        emit_av(*pend.pop(0))
                    at = at_ring[g % 3]
                    nc.scalar.activation(at[:, :], sg[:, :], ActF.Exp, scale=SCALE)
                    pend.append((g, at))
                for p in pend:
                    emit_av(*p)

                # normalize: obar = O / s (broadcast s first, then a
                # partition-parallel reciprocal -- a [1,512] DVE recip is ~3.4us)
                s_sb = out_pool.tile([1, NQ], FP, tag="r")
                nc.vector.tensor_copy(s_sb[:, :], o_ps[64:65, :])
                s_bc = out_pool.tile([64, NQ], FP, tag="sbc")
                nc.gpsimd.partition_broadcast(s_bc[:, :], s_sb[:, :])
                r_bc = out_pool.tile([64, NQ], FP, tag="rbc")
                nc.vector.reciprocal(r_bc[:, :], s_bc[:, :])
                ob = cpool.tile([64, NQ], FPR, tag=f"obar{h}")
                nc.vector.tensor_mul(ob[:, :], o_ps[0:64, :], r_bc[:, :])
                nc.sync.dma_start(out=obar_d[64 * h:64 * (h + 1), :], in_=ob[:, :])
                obars.append(ob)

            # ---------------- output projection (partial, no bias) ----------------
            for qb in range(4):
                for nh in range(2):
                    pp = psum_o.tile([128, 512], FP, tag="ot")
                    for h in range(HPC):
                        nc.tensor.matmul(pp[:, :],
                                         (obars[h][:, 128 * qb:128 * (qb + 1)]),
                                         (wp_h[h][:, 512 * nh:512 * (nh + 1)]),
                                         start=(h == 0), stop=(h == HPC - 1))
                    ot = out_pool.tile([128, 512], FP, tag="osb")
                    if nh == 0:
                        nc.vector.tensor_copy(ot[:, :], pp[:, :])
                    else:
                        nc.scalar.copy(ot[:, :], pp[:, :])
                    nc.sync.dma_start(
                        out=pout_d[128 * qb:128 * (qb + 1),
                                   512 * nh:512 * (nh + 1)],
                        in_=ot[:, :])

    nc.compile()
    return nc


def _get_nc():
    if "nc" not in _cache:
        _cache["nc"] = _build_program()
    return _cache["nc"]


def kernel(x, past_k, past_v, W_qkv, b_qkv, W_proj, b_proj, probe_idx, **kw):
    x = np.asarray(x)
    past_k = np.asarray(past_k)
    past_v = np.asarray(past_v)
    W_qkv = np.asarray(W_qkv)
    b_qkv = np.asarray(b_qkv)
    W_proj = np.asarray(W_proj)
    b_proj = np.asarray(b_proj)
    probe_idx = np.asarray(probe_idx)

    nc = _get_nc()
    x2 = np.ascontiguousarray(x.reshape(NQ, C).T)
    pkT_all = np.ascontiguousarray(past_k[0].transpose(0, 2, 1))
    pvp_all = np.ascontiguousarray(np.concatenate(
        [past_v[0], np.ones((H, LP, 1), np.float32)], axis=2))
    in_maps = []
    for g in range(NCORES):
        cols = slice(128 * g, 128 * (g + 1))
        # [3, 128(C within chunk), 8*128(cc, oc)]: wq[ocb, p, cc*128+oc]
        # = W_qkv[cc*128+p, ocb*C + g*128 + oc]
        wq_sl = np.stack([
            W_qkv[:, ocb * C + 128 * g: ocb * C + 128 * (g + 1)]
            .reshape(8, 128, 128).transpose(1, 0, 2).reshape(128, 1024)
            for ocb in range(3)])
        bq_sl = np.concatenate(
            [b_qkv[cols], b_qkv[C:][cols], b_qkv[2 * C:][cols]])
        in_maps.append({
            "x": x2,
            "wq": np.ascontiguousarray(wq_sl),
            "bq": np.ascontiguousarray(bq_sl),
            "wp": np.ascontiguousarray(W_proj[cols, :]),
            "pk": pkT_all[2 * g:2 * g + 2],
            "pv": pvp_all[2 * g:2 * g + 2],
        })

    res = run_bass_kernel_spmd(nc, in_maps, core_ids=list(range(NCORES)),
                               trace=bool(_cache.get("trace", False)))
    _cache["last_results"] = res

    out = np.zeros((NQ, C), dtype=np.float32)
    for g in range(NCORES):
        out += res.results[g]["pout"]
    out += b_proj[None, :]

    o_probe = np.empty((1, H, len(probe_idx), D), dtype=np.float32)
    for g in range(NCORES):
        ob = res.results[g]["obar"]          # [128, 512]: rows 64h+d
        for j in range(HPC):
            oT = ob[64 * j:64 * (j + 1), :]  # [D, NQ]
            o_probe[0, 2 * g + j] = oT[:, probe_idx].T
    return out.reshape(1, NQ, C), o_probe
